# revision 28
# baseline (speedup 1.0000x reference)
"""Trainium2 Bass kernel for a dense fp32 MultiHeadAttention layer.

Problem (hardcoded): B=4, T=S=2048, C=1024, 16 heads x 64 dims, fp32.
  q = query @ Wq.T + bq ; k,v likewise
  scores = (q k^T) * D**-0.5 (+ mask terms)
  out = softmax(scores) @ v -> reshape -> @ Wout.T + bout

Sharding over 8 NeuronCores: core c = (batch b = c//2, head-group g = c%2).
Each core handles one batch and 8 of the 16 heads (column-parallel q/k/v,
row-parallel out_proj; host sums the two partials per batch and adds the
bias terms bout + bv @ Wout.T, which commutes with softmax).

Differences from the v1 kernel (537us):
  - every matmul operand is bf16: fp32r matmuls are self-loading (weight
    load serialized inside each matmul, ~390ns/MM measured); bf16 gets a
    separate LDWEIGHTS that the PE reorder window hides, plus FWL.
  - the two heads of a pair are K=64 row-tiles at tile_position (0,0) /
    (64,0): with bf16 both score matmuls run CONCURRENTLY in the array.
  - software pipeline: block (pr) interleaves scores(pr) with PV(pr-1)
    per s-chunk so the scalar engine's exp (the other bottleneck, ~272us
    busy) always has PE work to overlap with. exp output dual-buffered
    at the pair level (bf16).
  - xq is staged in SBUF once per t-chunk (v1 re-streamed it twice).

Layout notes (per core):
  - host ships transposed activations xT (C, T) so projections need no
    on-device transposes.
  - scores are computed transposed (s on partitions, t free); v carries
    a ones column per head (width 65) so the PV matmul also produces the
    softmax denominator in row 64.
  - normalization: reciprocal of row 64, broadcast across partitions with
    a K=1 one-hot matmul, then one DVE multiply.
"""

import numpy as np
from ml_dtypes import bfloat16

import concourse.bass as bass
import concourse.mybir as mybir
import concourse.tile as tile
from concourse import bacc
from concourse.bass_utils import run_bass_kernel_spmd

# ---- problem constants ----
B, T, S, C = 4, 2048, 2048, 1024
H, D = 16, 64
NCORES = 8
F = 512            # per-core projection slice (8 heads x 64)
SCALE = D ** -0.5
P = 128
TCH = 512          # t-chunk (score free dim)
NTC = T // TCH     # 4
NSC = S // P       # 16 s-chunks
NFC = F // P       # 4 f-chunks (head pairs) per core
NCC = C // P       # 8 contraction chunks
HW = 65            # v width per head incl. ones column
NPR = NFC          # head pairs per core

FP32 = mybir.dt.float32
BF16 = mybir.dt.bfloat16

LAST_EXEC_NS = None
LAST_TRACE = None
LAST_NC = None
LAST_IN_MAPS = None


def build(use_mask: bool):
    nc = bacc.Bacc("TRN2", target_bir_lowering=False, debug=False,
                   num_devices=NCORES)

    # block layouts: every DMA reads a contiguous [P, 2, chunk] region so
    # per-partition lines are 2KB+ (bf16 halved line sizes; strided 1KB
    # lines starved phase 1 at ~90GB/s and delayed the first matmul 33us)
    NSW = S // TCH
    xq = nc.dram_tensor("xq", [NCC // 2, NTC, P, 2, TCH], BF16,
                        kind="ExternalInput")
    xk = nc.dram_tensor("xk", [NCC // 2, NSW, P, 2, TCH], BF16,
                        kind="ExternalInput")
    xv = nc.dram_tensor("xv", [NCC // 2, NSW, P, 2, TCH], BF16,
                        kind="ExternalInput")
    wq = nc.dram_tensor("wq", [NCC // 2, P, 2, F], BF16, kind="ExternalInput")
    wk = nc.dram_tensor("wk", [NCC // 2, P, 2, F], BF16, kind="ExternalInput")
    wv = nc.dram_tensor("wv", [NCC // 2, P, 2, F], BF16, kind="ExternalInput")
    wo = nc.dram_tensor("wo", [NFC, P, C], BF16, kind="ExternalInput")
    bqr = nc.dram_tensor("bqr", [P, NFC], FP32, kind="ExternalInput")
    bkr = nc.dram_tensor("bkr", [P, NFC], FP32, kind="ExternalInput")
    if use_mask:
        emask = nc.dram_tensor("emask", [S, T], FP32, kind="ExternalInput")
    out = nc.dram_tensor("out", [T, C], FP32, kind="ExternalOutput")

    with tile.TileContext(nc) as tc:
        with (
            tc.tile_pool(name="const", bufs=1) as cp,
            tc.tile_pool(name="qt", bufs=2) as qp,
            tc.tile_pool(name="exp", bufs=2) as ep,
            tc.tile_pool(name="xs", bufs=4) as xs,
            tc.tile_pool(name="xqs", bufs=1) as xqp,
            tc.tile_pool(name="ob", bufs=4) as mo,
            tc.tile_pool(name="bc", bufs=2) as bcp,
            tc.tile_pool(name="xkw", bufs=3) as xkwp,
            tc.tile_pool(name="xvw", bufs=2) as xvwp,
            tc.tile_pool(name="pscore", bufs=2, space="PSUM") as pscp,
            tc.tile_pool(name="ppv", bufs=2, space="PSUM") as ppvp,
            tc.tile_pool(name="pgen", bufs=2, space="PSUM") as pgp,
        ):
            wq_sb = cp.tile([P, NCC, F], BF16, tag="wq")
            wk_sb = cp.tile([P, NCC, F], BF16, tag="wk")
            wv_sb = cp.tile([P, NCC, F], BF16, tag="wv")
            wo_sb = cp.tile([P, NFC, C], BF16, tag="wo")
            bq_sb = cp.tile([P, NFC], FP32, tag="bq")
            bk_sb = cp.tile([P, NFC], FP32, tag="bk")
            rr32_sb = cp.tile([1, TCH], FP32, tag="rr32")
            rr32b_sb = cp.tile([1, TCH], FP32, tag="rr32b")
            kT_sb = cp.tile([P, NFC, S], BF16, tag="kT")
            v_sb = cp.tile([P, NSC, 8 * HW], BF16, tag="v")
            attnT = cp.tile([P, NFC, TCH], BF16, tag="attnT")

            for cp in range(NCC // 2):
                nc.sync.dma_start(
                    wk_sb[:].rearrange("p (cp i) f -> p cp i f", i=2)[:, cp],
                    wk[cp])
                nc.sync.dma_start(
                    wv_sb[:].rearrange("p (cp i) f -> p cp i f", i=2)[:, cp],
                    wv[cp])
            nc.sync.dma_start(bq_sb[:], bqr[:])
            nc.sync.dma_start(bk_sb[:], bkr[:])
            for cp in range(NCC // 2):
                nc.sync.dma_start(
                    wq_sb[:].rearrange("p (cp i) f -> p cp i f", i=2)[:, cp],
                    wq[cp])
            for dc in range(NFC):
                nc.sync.dma_start(wo_sb[:, dc, :], wo[dc])

            ones_dst = v_sb[:].rearrange("p s (h e) -> p s h e", e=HW)[:, :, :, D]
            nc.vector.memset(ones_dst, 1.0)

            # ---------------- phase 1 as filler units --------------------
            # k/v projections are split into per-(window, chunk) units and
            # interleaved into the first three blocks of the main loop so
            # the scalar engine's exp stream starts ~15us in instead of
            # idling behind a 70us serial prologue.
            kv_state = {}

            def stage_k(sw):
                t = xkwp.tile([P, NCC, TCH], BF16, tag="xkw", name="xkw")
                t4 = t[:].rearrange("p (cp i) t -> p cp i t", i=2)
                for cp in range(NCC // 2):
                    nc.sync.dma_start(t4[:, cp], xk[cp, sw])
                kv_state[("xk", sw)] = t

            def stage_v(sw):
                t = xvwp.tile([P, NCC, TCH], BF16, tag="xvw", name="xvw")
                t4 = t[:].rearrange("p (cp i) t -> p cp i t", i=2)
                for cp in range(NCC // 2):
                    nc.sync.dma_start(t4[:, cp], xv[cp, sw])
                kv_state[("xv", sw)] = t

            def kp(sw, fc):
                xkw = kv_state[("xk", sw)]
                ps = pgp.tile([P, TCH], FP32, tag="pgen", name="pkp")
                for cc in range(NCC):
                    nc.tensor.matmul(
                        ps[:], wk_sb[:, cc, fc * P:(fc + 1) * P],
                        xkw[:, cc, :],
                        start=(cc == 0), stop=(cc == NCC - 1))
                nc.vector.tensor_scalar_add(
                    kT_sb[:, fc, sw * TCH:(sw + 1) * TCH],
                    ps[:], bk_sb[:, fc:fc + 1])

            def vp(sw, ss):
                xvw = kv_state[("xv", sw)]
                ps = pgp.tile([P, TCH], FP32, tag="pgen", name="pvp")
                for cc in range(NCC):
                    nc.tensor.matmul(
                        ps[:], xvw[:, cc, ss * P:(ss + 1) * P],
                        wv_sb[:, cc, :],
                        start=(cc == 0), stop=(cc == NCC - 1))
                sc = sw * 4 + ss
                dst = v_sb[:, sc, :].rearrange("p (h e) -> p h e", e=HW)[:, :, 0:D]
                nc.vector.tensor_copy(dst, ps[:].rearrange("p (h e) -> p h e", e=D))

            UNIT = {"sk": stage_k, "sv": stage_v, "kp": kp, "vp": vp}
            # (block, batch) -> filler units. Fillers are emitted BEFORE the
            # batch's scores/PV, so a unit listed at the batch that first
            # consumes it is still program-order-correct.  Hard constraints:
            #  - blk b scores batch t read kT[:, pr=b, sc=2t..2t+1] -> the
            #    kp(sw=t//2, fc=b) unit must appear at or before (b, t)
            #  - PV in blk b+1 batch t reads v(2t..2t+1) -> vp likewise
            #  - stage_X(w) reuses a slot: every kp/vp of the window whose
            #    slot is being reused must already be emitted (xkw bufs=3,
            #    xvw bufs=2)
            FS = {
                (0, 0): [("sk", 1), ("kp", 0, 1)],
                (0, 1): [("kp", 1, 0), ("sk", 2)],
                (0, 2): [("kp", 0, 2), ("kp", 1, 1)],
                (0, 3): [("kp", 0, 3), ("sk", 3)],
                (0, 4): [("kp", 2, 0), ("kp", 1, 2)],
                (0, 5): [("kp", 3, 0), ("kp", 2, 1)],
                (0, 6): [("vp", 0, 0), ("kp", 1, 3)],
                (0, 7): [("vp", 0, 1), ("kp", 2, 2)],
                (1, 0): [("vp", 0, 2), ("vp", 0, 3)],
                (1, 1): [("sv", 1), ("kp", 2, 3)],
                (1, 2): [("vp", 1, 0), ("vp", 1, 1)],
                (1, 3): [("sv", 2), ("vp", 1, 2), ("vp", 1, 3)],
                (1, 4): [("vp", 2, 0), ("vp", 2, 1)],
                (1, 5): [("kp", 3, 1), ("vp", 2, 2), ("vp", 2, 3)],
                (1, 6): [("sv", 3), ("vp", 3, 0), ("vp", 3, 1)],
                (1, 7): [("kp", 3, 2), ("vp", 3, 2), ("vp", 3, 3)],
                (2, 0): [("kp", 3, 3)],
            }

            def emit_fillers(blk, batch):
                for u in FS.get((blk, batch), ()):
                    UNIT[u[0]](*u[1:])

            # prologue: first k window + first head pair's q projection
            stage_k(0)
            stage_v(0)
            kp(0, 0)

            # ---- q projection for one head pair ----
            xq_state = {}

            def qproj(tcx, pr):
                t0 = tcx * TCH
                if pr == 0:
                    xq_t = xqp.tile([P, NCC, TCH], BF16, tag="xq", name="xq_t")
                    xq_t4 = xq_t[:].rearrange("p (cp i) t -> p cp i t", i=2)
                    for cp in range(NCC // 2):
                        nc.sync.dma_start(xq_t4[:, cp], xq[cp, tcx])
                    xq_state["t"] = xq_t
                xq_t = xq_state["t"]
                psq = pgp.tile([P, TCH], FP32, tag="pgen", name="psq")
                for cc in range(NCC):
                    nc.tensor.matmul(
                        psq[:],
                        wq_sb[:, cc, pr * P:(pr + 1) * P],
                        xq_t[:, cc, :],
                        start=(cc == 0), stop=(cc == NCC - 1))
                qT = qp.tile([P, TCH], BF16, tag="qT", name="qT")
                nc.vector.tensor_scalar_add(qT[:], psq[:], bq_sb[:, pr:pr + 1])
                return qT

            def normalize(pr, ppvs):
                for h in range(2):
                    ppv = ppvs[h]
                    nc.vector.tensor_copy(rr32_sb[0:1, :], ppv[D:D + 1, :])
                    nc.vector.reciprocal_approx_fast(rr32b_sb[0:1, :],
                                                     rr32_sb[0:1, :])
                    # broadcast the reciprocal row across partitions on the
                    # (idle) GpSimd engine instead of a PE one-hot matmul
                    bc = bcp.tile([P, TCH], FP32, tag="bc", name="bc")
                    nc.gpsimd.partition_broadcast(bc[:], rr32b_sb[0:1, :])
                    dst = attnT[h * D:(h + 1) * D, pr, :]
                    nc.vector.tensor_copy(dst, ppv[0:D, :])
                    nc.vector.tensor_mul(dst, dst, bc[h * D:(h + 1) * D, :])

            def po_unit(tcx, j):
                # one (t-window, out-half) group of the out projection;
                # spread one per batch across the following block so the
                # tcx-boundary burst doesn't stall the next scores
                t0 = tcx * TCH
                tw, fh = divmod(j, 2)
                po = pgp.tile([P, TCH], FP32, tag="pgen", name="po")
                for dc in range(NFC):
                    nc.tensor.matmul(
                        po[:],
                        attnT[:, dc, tw * P:(tw + 1) * P],
                        wo_sb[:, dc, fh * TCH:(fh + 1) * TCH],
                        start=(dc == 0), stop=(dc == NFC - 1))
                ob = mo.tile([P, TCH], FP32, tag="ob")
                nc.vector.tensor_copy(ob[:], po[:])
                nc.sync.dma_start(
                    out[t0 + tw * P: t0 + (tw + 1) * P,
                        fh * TCH:(fh + 1) * TCH],
                    ob[:])

            def outproj(tcx):
                for j in range(8):
                    po_unit(tcx, j)

            def pv_mms(ppr, pexp, ppvs, sc):
                for h in range(2):
                    hh = ppr * 2 + h
                    nc.tensor.matmul(
                        ppvs[h][:],
                        v_sb[:, sc, hh * HW:(hh + 1) * HW],
                        pexp[:, sc, h * TCH:(h + 1) * TCH],
                        start=(sc == 0), stop=(sc == NSC - 1))

            # ---- main software-pipelined loop over blocks (tcx, pr) ----
            # block: scores+exp for pair pr, PV interleaved for the previous
            # block's pair, then that pair's normalization.
            prev = None   # (tcx, pr, expT)
            qT_cur = qproj(0, 0)
            qT_next = None
            po_pending = None   # tcx whose out-proj spreads over this block
            ppv_last = None     # self-PV accumulators (final block)
            blk = 0
            LAST = NTC * NPR - 1
            for tcx in range(NTC):
                for pr in range(NPR):
                    expT = ep.tile([P, NSC, 2 * TCH], BF16, tag="expT",
                                   name="expT")
                    ppvs = None
                    # 2-chunk batches: runs of same-shaped matmuls let the
                    # PE background-load the next weights during the current
                    # stream instead of context-switching every matmul.
                    for scp in range(NSC // 2):
                        emit_fillers(blk, scp)
                        scs = (2 * scp, 2 * scp + 1)
                        pscs = []
                        for sc in scs:
                            psc = pscp.tile([P, 2, TCH], FP32, tag="pscore",
                                            name="psc")
                            pscs.append(psc)
                            for h in range(2):
                                nc.tensor.matmul(
                                    psc[:, h, :],
                                    kT_sb[h * D:(h + 1) * D, pr,
                                          sc * P:(sc + 1) * P],
                                    qT_cur[h * D:(h + 1) * D, :],
                                    start=True, stop=True)
                        if prev is not None:
                            if scp == 0:
                                ppvs = [ppvp.tile([HW, TCH], FP32, tag="ppv",
                                                  name="ppv") for _ in range(2)]
                            for sc in scs:
                                pv_mms(prev[1], prev[2], ppvs, sc)
                        if blk == LAST and scp >= 1:
                            # final block: run this pair's own PV one batch
                            # behind exp, in the otherwise-idle pgen banks,
                            # so the drain has no PV left to do
                            if scp == 1:
                                ppv_last = [pgp.tile([P, TCH], FP32,
                                                     tag="pgen",
                                                     name="ppvl")[0:HW, :]
                                            for _ in range(2)]
                            for sc in (2 * scp - 2, 2 * scp - 1):
                                pv_mms(pr, expT, ppv_last, sc)
                        if po_pending is not None:
                            po_unit(po_pending, scp)
                            if scp == NSC // 2 - 1:
                                po_pending = None
                        if scp == 6:
                            npr = (pr + 1) % NPR
                            ntcx = tcx + (1 if npr == 0 else 0)
                            if ntcx < NTC:
                                qT_next = qproj(ntcx, npr)
                        for sc, psc in zip(scs, pscs):
                            nc.scalar.activation(
                                expT[:, sc, :],
                                psc[:].rearrange("p a b -> p (a b)"),
                                mybir.ActivationFunctionType.Exp, scale=SCALE)
                        if use_mask:
                            for sc in scs:
                                em_t = xs.tile([P, TCH], FP32, tag="emk")
                                nc.sync.dma_start(
                                    em_t[:],
                                    emask[sc * P:(sc + 1) * P,
                                          tcx * TCH:(tcx + 1) * TCH])
                                for h in range(2):
                                    nc.vector.tensor_mul(
                                        expT[:, sc, h * TCH:(h + 1) * TCH],
                                        expT[:, sc, h * TCH:(h + 1) * TCH],
                                        em_t[:])
                    if qT_next is not None:
                        qT_cur = qT_next
                    if prev is not None:
                        normalize(prev[1], ppvs)
                        if prev[1] == NPR - 1:
                            po_pending = prev[0]
                    prev = (tcx, pr, expT)
                    blk += 1

            # drain: last pair's final PV chunks + normalize + out proj
            ptcx, ppr, pexp = prev
            for sc in (NSC - 2, NSC - 1):
                pv_mms(ppr, pexp, ppv_last, sc)
            normalize(ppr, ppv_last)
            outproj(ptcx)

    nc.compile()
    return nc


_CACHE = {}


def _get(use_mask: bool):
    if use_mask not in _CACHE:
        _CACHE[use_mask] = build(use_mask)
    return _CACHE[use_mask]


def kernel(query, key, value, attn_mask, key_padding_mask,
           Wq, bq, Wk, bk, Wv, bv, Wout, bout):
    global LAST_EXEC_NS, LAST_TRACE
    query = np.asarray(query, np.float32)
    key = np.asarray(key, np.float32)
    value = np.asarray(value, np.float32)
    attn_mask = np.asarray(attn_mask, np.float32)
    key_padding_mask = np.asarray(key_padding_mask)
    Wq, bq = np.asarray(Wq, np.float32), np.asarray(bq, np.float32)
    Wk, bk = np.asarray(Wk, np.float32), np.asarray(bk, np.float32)
    Wv, bv = np.asarray(Wv, np.float32), np.asarray(bv, np.float32)
    Wout, bout = np.asarray(Wout, np.float32), np.asarray(bout, np.float32)

    use_mask = bool(np.any(attn_mask)) or bool(np.any(key_padding_mask))
    nc = _get(use_mask)

    def xblocks(xT, nt):
        # (C, T) -> [cp, tcx, p, i, tt] contiguous bf16 blocks
        a = xT.reshape(NCC // 2, 2, P, nt, -1).transpose(0, 3, 2, 1, 4)
        return np.ascontiguousarray(a.astype(bfloat16))

    def wblocks(wT):
        # (C, F) -> [cp, p, i, f]
        a = wT.reshape(NCC // 2, 2, P, F).transpose(0, 2, 1, 3)
        return np.ascontiguousarray(a.astype(bfloat16))

    in_maps = []
    for c in range(NCORES):
        b, g = divmod(c, 2)
        gs = g * F
        im = {
            "xq": xblocks(query[b].T, NTC),
            "xk": xblocks(key[b].T, S // TCH),
            "xv": xblocks(value[b].T, S // TCH),
            "wq": wblocks(Wq[gs:gs + F, :].T),
            "wk": wblocks(Wk[gs:gs + F, :].T),
            "wv": wblocks(Wv[gs:gs + F, :].T),
            "wo": np.ascontiguousarray(
                Wout[:, gs:gs + F].T.reshape(NFC, P, C).astype(bfloat16)),
            "bqr": np.ascontiguousarray(bq[gs:gs + F].reshape(NFC, P).T),
            "bkr": np.ascontiguousarray(bk[gs:gs + F].reshape(NFC, P).T),
        }
        if use_mask:
            m = attn_mask.T.astype(np.float64).copy()
            m[key_padding_mask[b], :] = -np.inf
            im["emask"] = np.exp(m).astype(np.float32)
        in_maps.append(im)

    global LAST_NC, LAST_IN_MAPS
    LAST_NC, LAST_IN_MAPS = nc, in_maps
    res = run_bass_kernel_spmd(nc, in_maps, list(range(NCORES)))
    LAST_EXEC_NS = res.exec_time_ns
    LAST_TRACE = res.instructions_and_trace[1] if res.instructions_and_trace else None
    globals()["LAST_INSTS"] = (res.instructions_and_trace[0]
                               if res.instructions_and_trace else None)

    extra = (bv @ Wout.T + bout).astype(np.float32)
    outp = np.empty((B, T, C), np.float32)
    for b in range(B):
        outp[b] = res.results[2 * b]["out"] + res.results[2 * b + 1]["out"] + extra
    return outp


# revision 31
# speedup vs baseline: 1.0141x; 1.0141x over previous
"""Trainium2 Bass kernel for a dense fp32 MultiHeadAttention layer.

Problem (hardcoded): B=4, T=S=2048, C=1024, 16 heads x 64 dims, fp32.
  q = query @ Wq.T + bq ; k,v likewise
  scores = (q k^T) * D**-0.5 (+ mask terms)
  out = softmax(scores) @ v -> reshape -> @ Wout.T + bout

Sharding over 8 NeuronCores: core c = (batch b = c//2, head-group g = c%2).
Each core handles one batch and 8 of the 16 heads (column-parallel q/k/v,
row-parallel out_proj; host sums the two partials per batch and adds the
bias terms bout + bv @ Wout.T, which commutes with softmax).

Differences from the v1 kernel (537us):
  - every matmul operand is bf16: fp32r matmuls are self-loading (weight
    load serialized inside each matmul, ~390ns/MM measured); bf16 gets a
    separate LDWEIGHTS that the PE reorder window hides, plus FWL.
  - the two heads of a pair are K=64 row-tiles at tile_position (0,0) /
    (64,0): with bf16 both score matmuls run CONCURRENTLY in the array.
  - software pipeline: block (pr) interleaves scores(pr) with PV(pr-1)
    per s-chunk so the scalar engine's exp (the other bottleneck, ~272us
    busy) always has PE work to overlap with. exp output dual-buffered
    at the pair level (bf16).
  - xq is staged in SBUF once per t-chunk (v1 re-streamed it twice).

Layout notes (per core):
  - host ships transposed activations xT (C, T) so projections need no
    on-device transposes.
  - scores are computed transposed (s on partitions, t free); v carries
    a ones column per head (width 65) so the PV matmul also produces the
    softmax denominator in row 64.
  - normalization: reciprocal of row 64, broadcast across partitions with
    a K=1 one-hot matmul, then one DVE multiply.
"""

import numpy as np
from ml_dtypes import bfloat16

import concourse.bass as bass
import concourse.mybir as mybir
import concourse.tile as tile
from concourse import bacc
from concourse.bass_utils import run_bass_kernel_spmd

# ---- problem constants ----
B, T, S, C = 4, 2048, 2048, 1024
H, D = 16, 64
NCORES = 8
F = 512            # per-core projection slice (8 heads x 64)
SCALE = D ** -0.5
P = 128
TCH = 512          # t-chunk (score free dim)
NTC = T // TCH     # 4
NSC = S // P       # 16 s-chunks
NFC = F // P       # 4 f-chunks (head pairs) per core
NCC = C // P       # 8 contraction chunks
HW = 65            # v width per head incl. ones column
NPR = NFC          # head pairs per core

FP32 = mybir.dt.float32
BF16 = mybir.dt.bfloat16

LAST_EXEC_NS = None
LAST_TRACE = None
LAST_NC = None
LAST_IN_MAPS = None


def build(use_mask: bool):
    nc = bacc.Bacc("TRN2", target_bir_lowering=False, debug=False,
                   num_devices=NCORES)

    # block layouts: every DMA reads a contiguous [P, 2, chunk] region so
    # per-partition lines are 2KB+ (bf16 halved line sizes; strided 1KB
    # lines starved phase 1 at ~90GB/s and delayed the first matmul 33us)
    NSW = S // TCH
    xq = nc.dram_tensor("xq", [NCC // 2, NTC, P, 2, TCH], BF16,
                        kind="ExternalInput")
    xk = nc.dram_tensor("xk", [NCC // 2, NSW, P, 2, TCH], BF16,
                        kind="ExternalInput")
    xv = nc.dram_tensor("xv", [NCC // 2, NSW, P, 2, TCH], BF16,
                        kind="ExternalInput")
    wq = nc.dram_tensor("wq", [NCC // 2, P, 2, F], BF16, kind="ExternalInput")
    wk = nc.dram_tensor("wk", [NCC // 2, P, 2, F], BF16, kind="ExternalInput")
    wv = nc.dram_tensor("wv", [NCC // 2, P, 2, F], BF16, kind="ExternalInput")
    wo = nc.dram_tensor("wo", [NFC, P, C], BF16, kind="ExternalInput")
    bqr = nc.dram_tensor("bqr", [P, NFC], FP32, kind="ExternalInput")
    bkr = nc.dram_tensor("bkr", [P, NFC], FP32, kind="ExternalInput")
    if use_mask:
        emask = nc.dram_tensor("emask", [S, T], FP32, kind="ExternalInput")
    out = nc.dram_tensor("out", [T, C], FP32, kind="ExternalOutput")

    with tile.TileContext(nc) as tc:
        with (
            tc.tile_pool(name="const", bufs=1) as cp,
            tc.tile_pool(name="qt", bufs=2) as qp,
            tc.tile_pool(name="exp", bufs=2) as ep,
            tc.tile_pool(name="xs", bufs=4) as xs,
            tc.tile_pool(name="xqs", bufs=1) as xqp,
            tc.tile_pool(name="ob", bufs=4) as mo,
            tc.tile_pool(name="bc", bufs=2) as bcp,
            tc.tile_pool(name="xkw", bufs=3) as xkwp,
            tc.tile_pool(name="xvw", bufs=2) as xvwp,
            tc.tile_pool(name="pscore", bufs=2, space="PSUM") as pscp,
            tc.tile_pool(name="ppv", bufs=2, space="PSUM") as ppvp,
            tc.tile_pool(name="pgen", bufs=2, space="PSUM") as pgp,
        ):
            wq_sb = cp.tile([P, NCC, F], BF16, tag="wq")
            wk_sb = cp.tile([P, NCC, F], BF16, tag="wk")
            wv_sb = cp.tile([P, NCC, F], BF16, tag="wv")
            wo_sb = cp.tile([P, NFC, C], BF16, tag="wo")
            bq_sb = cp.tile([P, NFC], FP32, tag="bq")
            bk_sb = cp.tile([P, NFC], FP32, tag="bk")
            rr32_sb = cp.tile([1, TCH], FP32, tag="rr32")
            rr32b_sb = cp.tile([1, TCH], FP32, tag="rr32b")
            kT_sb = cp.tile([P, NFC, S], BF16, tag="kT")
            v_sb = cp.tile([P, NSC, 8 * HW], BF16, tag="v")
            attnT = cp.tile([P, NFC, TCH], BF16, tag="attnT")

            # only what the FIRST matmul needs goes out first (wk + biases;
            # the first xk window is staged right after): the remaining 5MB
            # of weights/windows used to delay the first matmul to ~24us
            for cp in range(NCC // 2):
                nc.sync.dma_start(
                    wk_sb[:].rearrange("p (cp i) f -> p cp i f", i=2)[:, cp],
                    wk[cp])
            nc.sync.dma_start(bk_sb[:], bkr[:])
            nc.sync.dma_start(bq_sb[:], bqr[:])

            ones_dst = v_sb[:].rearrange("p s (h e) -> p s h e", e=HW)[:, :, :, D]
            nc.vector.memset(ones_dst, 1.0)

            # ---------------- phase 1 as filler units --------------------
            # k/v projections are split into per-(window, chunk) units and
            # interleaved into the first three blocks of the main loop so
            # the scalar engine's exp stream starts ~15us in instead of
            # idling behind a 70us serial prologue.
            kv_state = {}

            def stage_k(sw):
                t = xkwp.tile([P, NCC, TCH], BF16, tag="xkw", name="xkw")
                t4 = t[:].rearrange("p (cp i) t -> p cp i t", i=2)
                for cp in range(NCC // 2):
                    nc.sync.dma_start(t4[:, cp], xk[cp, sw])
                kv_state[("xk", sw)] = t

            def stage_v(sw):
                t = xvwp.tile([P, NCC, TCH], BF16, tag="xvw", name="xvw")
                t4 = t[:].rearrange("p (cp i) t -> p cp i t", i=2)
                for cp in range(NCC // 2):
                    nc.sync.dma_start(t4[:, cp], xv[cp, sw])
                kv_state[("xv", sw)] = t

            def kp(sw, fc):
                xkw = kv_state[("xk", sw)]
                ps = pgp.tile([P, TCH], FP32, tag="pgen", name="pkp")
                for cc in range(NCC):
                    nc.tensor.matmul(
                        ps[:], wk_sb[:, cc, fc * P:(fc + 1) * P],
                        xkw[:, cc, :],
                        start=(cc == 0), stop=(cc == NCC - 1))
                nc.vector.tensor_scalar_add(
                    kT_sb[:, fc, sw * TCH:(sw + 1) * TCH],
                    ps[:], bk_sb[:, fc:fc + 1])

            def vp(sw, ss):
                xvw = kv_state[("xv", sw)]
                ps = pgp.tile([P, TCH], FP32, tag="pgen", name="pvp")
                for cc in range(NCC):
                    nc.tensor.matmul(
                        ps[:], xvw[:, cc, ss * P:(ss + 1) * P],
                        wv_sb[:, cc, :],
                        start=(cc == 0), stop=(cc == NCC - 1))
                sc = sw * 4 + ss
                dst = v_sb[:, sc, :].rearrange("p (h e) -> p h e", e=HW)[:, :, 0:D]
                nc.vector.tensor_copy(dst, ps[:].rearrange("p (h e) -> p h e", e=D))

            UNIT = {"sk": stage_k, "sv": stage_v, "kp": kp, "vp": vp}
            # (block, batch) -> filler units. Fillers are emitted BEFORE the
            # batch's scores/PV, so a unit listed at the batch that first
            # consumes it is still program-order-correct.  Hard constraints:
            #  - blk b scores batch t read kT[:, pr=b, sc=2t..2t+1] -> the
            #    kp(sw=t//2, fc=b) unit must appear at or before (b, t)
            #  - PV in blk b+1 batch t reads v(2t..2t+1) -> vp likewise
            #  - stage_X(w) reuses a slot: every kp/vp of the window whose
            #    slot is being reused must already be emitted (xkw bufs=3,
            #    xvw bufs=2)
            FS = {
                (0, 0): [("sk", 1), ("kp", 0, 1)],
                (0, 1): [("kp", 1, 0), ("sk", 2)],
                (0, 2): [("kp", 0, 2), ("kp", 1, 1)],
                (0, 3): [("kp", 0, 3), ("sk", 3)],
                (0, 4): [("kp", 2, 0), ("kp", 1, 2)],
                (0, 5): [("kp", 3, 0), ("kp", 2, 1)],
                (0, 6): [("vp", 0, 0), ("kp", 1, 3)],
                (0, 7): [("vp", 0, 1), ("kp", 2, 2)],
                (1, 0): [("vp", 0, 2), ("vp", 0, 3)],
                (1, 1): [("sv", 1), ("kp", 2, 3)],
                (1, 2): [("vp", 1, 0), ("vp", 1, 1)],
                (1, 3): [("sv", 2), ("vp", 1, 2), ("vp", 1, 3)],
                (1, 4): [("vp", 2, 0), ("vp", 2, 1)],
                (1, 5): [("kp", 3, 1), ("vp", 2, 2), ("vp", 2, 3)],
                (1, 6): [("sv", 3), ("vp", 3, 0), ("vp", 3, 1)],
                (1, 7): [("kp", 3, 2), ("vp", 3, 2), ("vp", 3, 3)],
                (2, 0): [("kp", 3, 3)],
            }

            def emit_fillers(blk, batch):
                for u in FS.get((blk, batch), ()):
                    UNIT[u[0]](*u[1:])

            # prologue: first k window + first head pair's q projection
            stage_k(0)
            kp(0, 0)
            # deferred DMAs, in consumption order: wq feeds qproj(0,0) next,
            # wv + the first xv window feed the vp fillers (~block 0 batch
            # 6), wo is not read until the first out-projection (block 5)
            for cpr in range(NCC // 2):
                nc.sync.dma_start(
                    wq_sb[:].rearrange("p (cp i) f -> p cp i f", i=2)[:, cpr],
                    wq[cpr])
            for cpr in range(NCC // 2):
                nc.sync.dma_start(
                    wv_sb[:].rearrange("p (cp i) f -> p cp i f", i=2)[:, cpr],
                    wv[cpr])
            stage_v(0)
            for dc in range(NFC):
                nc.sync.dma_start(wo_sb[:, dc, :], wo[dc])

            # ---- q projection for one head pair ----
            xq_state = {}

            def qproj(tcx, pr):
                t0 = tcx * TCH
                if pr == 0:
                    xq_t = xqp.tile([P, NCC, TCH], BF16, tag="xq", name="xq_t")
                    xq_t4 = xq_t[:].rearrange("p (cp i) t -> p cp i t", i=2)
                    for cp in range(NCC // 2):
                        nc.sync.dma_start(xq_t4[:, cp], xq[cp, tcx])
                    xq_state["t"] = xq_t
                xq_t = xq_state["t"]
                psq = pgp.tile([P, TCH], FP32, tag="pgen", name="psq")
                for cc in range(NCC):
                    nc.tensor.matmul(
                        psq[:],
                        wq_sb[:, cc, pr * P:(pr + 1) * P],
                        xq_t[:, cc, :],
                        start=(cc == 0), stop=(cc == NCC - 1))
                qT = qp.tile([P, TCH], BF16, tag="qT", name="qT")
                nc.vector.tensor_scalar_add(qT[:], psq[:], bq_sb[:, pr:pr + 1])
                return qT

            def normalize(pr, ppvs):
                for h in range(2):
                    ppv = ppvs[h]
                    nc.vector.tensor_copy(rr32_sb[0:1, :], ppv[D:D + 1, :])
                    nc.vector.reciprocal_approx_fast(rr32b_sb[0:1, :],
                                                     rr32_sb[0:1, :])
                    # broadcast the reciprocal row across partitions on the
                    # (idle) GpSimd engine instead of a PE one-hot matmul
                    bc = bcp.tile([P, TCH], FP32, tag="bc", name="bc")
                    nc.gpsimd.partition_broadcast(bc[:], rr32b_sb[0:1, :])
                    dst = attnT[h * D:(h + 1) * D, pr, :]
                    nc.vector.tensor_copy(dst, ppv[0:D, :])
                    nc.vector.tensor_mul(dst, dst, bc[h * D:(h + 1) * D, :])

            def outproj(tcx):
                t0 = tcx * TCH
                for tw in range(TCH // P):
                    for fh in range(2):
                        po = pgp.tile([P, TCH], FP32, tag="pgen", name="po")
                        for dc in range(NFC):
                            nc.tensor.matmul(
                                po[:],
                                attnT[:, dc, tw * P:(tw + 1) * P],
                                wo_sb[:, dc, fh * TCH:(fh + 1) * TCH],
                                start=(dc == 0), stop=(dc == NFC - 1))
                        ob = mo.tile([P, TCH], FP32, tag="ob")
                        nc.vector.tensor_copy(ob[:], po[:])
                        nc.sync.dma_start(
                            out[t0 + tw * P: t0 + (tw + 1) * P,
                                fh * TCH:(fh + 1) * TCH],
                            ob[:])

            def pv_mms(ppr, pexp, ppvs, sc):
                for h in range(2):
                    hh = ppr * 2 + h
                    nc.tensor.matmul(
                        ppvs[h][:],
                        v_sb[:, sc, hh * HW:(hh + 1) * HW],
                        pexp[:, sc, h * TCH:(h + 1) * TCH],
                        start=(sc == 0), stop=(sc == NSC - 1))

            # ---- main software-pipelined loop over blocks (tcx, pr) ----
            # block: scores+exp for pair pr, PV interleaved for the previous
            # block's pair, then that pair's normalization.
            prev = None   # (tcx, pr, expT)
            qT_cur = qproj(0, 0)
            blk = 0
            for tcx in range(NTC):
                for pr in range(NPR):
                    expT = ep.tile([P, NSC, 2 * TCH], BF16, tag="expT",
                                   name="expT")
                    ppvs = None
                    # 2-chunk batches: runs of same-shaped matmuls let the
                    # PE background-load the next weights during the current
                    # stream instead of context-switching every matmul.
                    for scp in range(NSC // 2):
                        emit_fillers(blk, scp)
                        scs = (2 * scp, 2 * scp + 1)
                        pscs = []
                        for sc in scs:
                            psc = pscp.tile([P, 2, TCH], FP32, tag="pscore",
                                            name="psc")
                            pscs.append(psc)
                            for h in range(2):
                                nc.tensor.matmul(
                                    psc[:, h, :],
                                    kT_sb[h * D:(h + 1) * D, pr,
                                          sc * P:(sc + 1) * P],
                                    qT_cur[h * D:(h + 1) * D, :],
                                    start=True, stop=True)
                        if prev is not None:
                            if scp == 0:
                                ppvs = [ppvp.tile([HW, TCH], FP32, tag="ppv",
                                                  name="ppv") for _ in range(2)]
                            for sc in scs:
                                pv_mms(prev[1], prev[2], ppvs, sc)
                        for sc, psc in zip(scs, pscs):
                            nc.scalar.activation(
                                expT[:, sc, :],
                                psc[:].rearrange("p a b -> p (a b)"),
                                mybir.ActivationFunctionType.Exp, scale=SCALE)
                        if use_mask:
                            for sc in scs:
                                em_t = xs.tile([P, TCH], FP32, tag="emk")
                                nc.sync.dma_start(
                                    em_t[:],
                                    emask[sc * P:(sc + 1) * P,
                                          tcx * TCH:(tcx + 1) * TCH])
                                for h in range(2):
                                    nc.vector.tensor_mul(
                                        expT[:, sc, h * TCH:(h + 1) * TCH],
                                        expT[:, sc, h * TCH:(h + 1) * TCH],
                                        em_t[:])
                    # next block's q projection (fills PE while exp drains)
                    npr = (pr + 1) % NPR
                    ntcx = tcx + (1 if npr == 0 else 0)
                    if ntcx < NTC:
                        qT_cur = qproj(ntcx, npr)
                    if prev is not None:
                        normalize(prev[1], ppvs)
                        if prev[1] == NPR - 1:
                            outproj(prev[0])
                    prev = (tcx, pr, expT)
                    blk += 1

            # drain: PV + normalize + outproj for the last pair
            ptcx, ppr, pexp = prev
            ppvs = [ppvp.tile([HW, TCH], FP32, tag="ppv", name="ppv")
                    for _ in range(2)]
            for sc in range(NSC):
                pv_mms(ppr, pexp, ppvs, sc)
            normalize(ppr, ppvs)
            outproj(ptcx)

    nc.compile()
    return nc


_CACHE = {}


def _get(use_mask: bool):
    if use_mask not in _CACHE:
        _CACHE[use_mask] = build(use_mask)
    return _CACHE[use_mask]


def kernel(query, key, value, attn_mask, key_padding_mask,
           Wq, bq, Wk, bk, Wv, bv, Wout, bout):
    global LAST_EXEC_NS, LAST_TRACE
    query = np.asarray(query, np.float32)
    key = np.asarray(key, np.float32)
    value = np.asarray(value, np.float32)
    attn_mask = np.asarray(attn_mask, np.float32)
    key_padding_mask = np.asarray(key_padding_mask)
    Wq, bq = np.asarray(Wq, np.float32), np.asarray(bq, np.float32)
    Wk, bk = np.asarray(Wk, np.float32), np.asarray(bk, np.float32)
    Wv, bv = np.asarray(Wv, np.float32), np.asarray(bv, np.float32)
    Wout, bout = np.asarray(Wout, np.float32), np.asarray(bout, np.float32)

    use_mask = bool(np.any(attn_mask)) or bool(np.any(key_padding_mask))
    nc = _get(use_mask)

    def xblocks(xT, nt):
        # (C, T) -> [cp, tcx, p, i, tt] contiguous bf16 blocks
        a = xT.reshape(NCC // 2, 2, P, nt, -1).transpose(0, 3, 2, 1, 4)
        return np.ascontiguousarray(a.astype(bfloat16))

    def wblocks(wT):
        # (C, F) -> [cp, p, i, f]
        a = wT.reshape(NCC // 2, 2, P, F).transpose(0, 2, 1, 3)
        return np.ascontiguousarray(a.astype(bfloat16))

    in_maps = []
    for c in range(NCORES):
        b, g = divmod(c, 2)
        gs = g * F
        im = {
            "xq": xblocks(query[b].T, NTC),
            "xk": xblocks(key[b].T, S // TCH),
            "xv": xblocks(value[b].T, S // TCH),
            "wq": wblocks(Wq[gs:gs + F, :].T),
            "wk": wblocks(Wk[gs:gs + F, :].T),
            "wv": wblocks(Wv[gs:gs + F, :].T),
            "wo": np.ascontiguousarray(
                Wout[:, gs:gs + F].T.reshape(NFC, P, C).astype(bfloat16)),
            "bqr": np.ascontiguousarray(bq[gs:gs + F].reshape(NFC, P).T),
            "bkr": np.ascontiguousarray(bk[gs:gs + F].reshape(NFC, P).T),
        }
        if use_mask:
            m = attn_mask.T.astype(np.float64).copy()
            m[key_padding_mask[b], :] = -np.inf
            im["emask"] = np.exp(m).astype(np.float32)
        in_maps.append(im)

    global LAST_NC, LAST_IN_MAPS
    LAST_NC, LAST_IN_MAPS = nc, in_maps
    res = run_bass_kernel_spmd(nc, in_maps, list(range(NCORES)))
    LAST_EXEC_NS = res.exec_time_ns
    LAST_TRACE = res.instructions_and_trace[1] if res.instructions_and_trace else None
    globals()["LAST_INSTS"] = (res.instructions_and_trace[0]
                               if res.instructions_and_trace else None)

    extra = (bv @ Wout.T + bout).astype(np.float32)
    outp = np.empty((B, T, C), np.float32)
    for b in range(B):
        outp[b] = res.results[2 * b]["out"] + res.results[2 * b + 1]["out"] + extra
    return outp
